# revision 5
# baseline (speedup 1.0000x reference)
"""Cross-attention kernel for Trainium2, distributed over 8 NeuronCores.

Sharding: batch x head parallel. Cores 0-3 handle batch 0, cores 4-7 batch 1.
Within a team of 4, core r handles heads 4r..4r+3 (channel slice 256r..256r+256).

Per core:
  - KV projection for its 256 k-channels + 256 v-channels (tensor parallel,
    contraction over full D with host-pretransposed context/W_kv)
  - k and q LayerNorm stats: partial (sum, sumsq) per row over the core's
    256-channel slice; two 16KB AllReduces within the team (tiles 0-7 launch
    mid-projection so the latency hides under the rest of stage A)
  - k/q LN apply + transpose pipelined into the attention loop (per-tile,
    just before first use)
  - attention for its 4 heads, computed transposed (simT[j,i] = k.q) with
    softmax denominators from an appended ones-column in v (no max
    subtraction: |sim*scale| <= ~6 for this problem, exp stays in fp32 range)
  - Exp runs 1024 wide (two k-tiles per activation instruction)
  - output projection: partial y over the core's 256 channels for all 2048
    queries (W_out row-slice), per-512-row-block ReduceScatter within the
    team sums the partials; each core keeps a disjoint 128-row quarter of
    each block. Host assembles; bias is folded in by passing b_out to team
    rank 0 and zeros to ranks 1-3.
"""

import numpy as np

import concourse.bass as bass
import concourse.mybir as mybir
import concourse.tile as tile
from concourse import bacc
from concourse.bass_utils import run_bass_kernel_spmd
from concourse.masks import make_identity

B, NQ, NK, D, H, DH = 2, 2048, 2048, 1024, 16, 64
NCORES = 8
TEAM = 4
HPC = 4            # heads per core
DSL = HPC * DH     # 256: per-core channel slice
EPS = 1e-6
SCALE = DH ** -0.5
GROUPS = [[0, 1, 2, 3], [4, 5, 6, 7]]
FP32 = mybir.dt.float32
FP32R = mybir.dt.float32r
NT = NQ // 128     # 16 row tiles
KC = D // 128      # 8 contraction chunks (kv proj)
KCO = DSL // 128   # 2 contraction chunks (out proj)
NBLK = 4           # 512-query blocks
TPB = NT // NBLK   # 4 row tiles per block

_CACHE: dict = {}
MOCK_COLL = False  # replace collectives with local DMA (for TimelineSim)


def _bcast_ap(t, parts):
    ap = t.ap() if hasattr(t, "ap") and not isinstance(t, bass.AP) else t
    return bass.AP(tensor=ap.tensor, offset=ap.offset,
                   ap=[[0, parts]] + list(ap.ap))


def _build():
    nc = bacc.Bacc("TRN2", target_bir_lowering=False, debug=False,
                   num_devices=NCORES)
    x_s = nc.declare_dram_parameter("x_s", [NQ, DSL], FP32, isOutput=False)
    ctxT = nc.declare_dram_parameter("ctxT", [D, NK], FP32, isOutput=False)
    wkvT = nc.declare_dram_parameter("wkvT", [D, 2 * DSL], FP32, isOutput=False)
    woutT = nc.declare_dram_parameter("woutT", [DSL, D], FP32, isOutput=False)
    bout = nc.declare_dram_parameter("bout", [D], FP32, isOutput=False)
    gq_s = nc.declare_dram_parameter("gq_s", [DSL], FP32, isOutput=False)
    bq_s = nc.declare_dram_parameter("bq_s", [DSL], FP32, isOutput=False)
    gk_s = nc.declare_dram_parameter("gk_s", [DSL], FP32, isOutput=False)
    bk_s = nc.declare_dram_parameter("bk_s", [DSL], FP32, isOutput=False)
    y_out = nc.declare_dram_parameter("y_out", [NBLK, 128, D], FP32,
                                      isOutput=True)

    stats_dram = [nc.dram_tensor(f"stats_dram{i}", [128, 32], FP32)
                  for i in range(2)]
    statsr_dram = [nc.dram_tensor(f"statsr_dram{i}", [128, 32], FP32)
                   for i in range(2)]
    ypart_blk = [nc.dram_tensor(f"ypart_blk{i}", [512, D], FP32)
                 for i in range(NBLK)]
    yrs_blk = [nc.dram_tensor(f"yrs_blk{i}", [128, D], FP32)
               for i in range(NBLK)]

    ctxT_r = ctxT.ap().rearrange("(k p) m -> p k m", p=128)    # [128, 8, NK]
    wkvT_r = wkvT.ap().rearrange("(k p) n -> p k n", p=128)    # [128, 8, 512]
    woutT_r = woutT.ap().rearrange("(k p) n -> p k n", p=128)  # [128, 2, D]

    with tile.TileContext(nc) as tc:
        with (
            tc.tile_pool(name="singles", bufs=1) as singles,
            tc.tile_pool(name="ld", bufs=3) as ld,
            tc.tile_pool(name="work", bufs=3) as work,
            tc.tile_pool(name="psmm", bufs=2, space="PSUM") as psmm,
            tc.tile_pool(name="pssim", bufs=2, space="PSUM") as pssim,
            tc.tile_pool(name="psout", bufs=2, space="PSUM") as psout,
        ):
            # --- persistent sbuf; small params + wkv first, wout later ---
            ident = singles.tile([128, 128], FP32)
            make_identity(nc, ident)
            eps_sb = singles.tile([128, 1], FP32)
            nc.vector.memset(eps_sb, EPS)

            def _col_ap(param, cb):
                ap = param.ap()
                return bass.AP(tensor=ap.tensor, offset=128 * cb,
                               ap=[[1, 128], [1, 1]])

            gqT = [singles.tile([128, 1], FP32, name=f"gqT{cb}") for cb in range(2)]
            bqT = [singles.tile([128, 1], FP32, name=f"bqT{cb}") for cb in range(2)]
            gkT = [singles.tile([128, 1], FP32, name=f"gkT{cb}") for cb in range(2)]
            bkT = [singles.tile([128, 1], FP32, name=f"bkT{cb}") for cb in range(2)]
            for cb in range(2):
                nc.sync.dma_start(out=gqT[cb], in_=_col_ap(gq_s, cb))
                nc.sync.dma_start(out=bqT[cb], in_=_col_ap(bq_s, cb))
                nc.sync.dma_start(out=gkT[cb], in_=_col_ap(gk_s, cb))
                nc.sync.dma_start(out=bkT[cb], in_=_col_ap(bk_s, cb))
            bout_b = singles.tile([128, D], FP32)
            nc.sync.dma_start(out=bout_b, in_=_bcast_ap(bout, 128))
            wkv_sb = singles.tile([128, KC, 2 * DSL], FP32R)
            nc.sync.dma_start(out=wkv_sb, in_=wkvT_r.bitcast(FP32R))
            wout_sb = singles.tile([128, KCO, D], FP32R)  # loaded after stage A

            x_nat = singles.tile([128, NT, DSL], FP32)
            k_nat = singles.tile([128, NT, DSL], FP32)
            vh_sb = [singles.tile([128, NT, DH + 2], FP32R, tag=f"vh{h}",
                                  name=f"vh{h}") for h in range(HPC)]
            for h in range(HPC):
                nc.vector.memset(vh_sb[h][:, :, DH:DH + 1].bitcast(FP32), 1.0)
                nc.vector.memset(vh_sb[h][:, :, DH + 1:DH + 2].bitcast(FP32), 0.0)
            qT_sb = [singles.tile([128, NT, 128], FP32R, tag=f"qT{cb}",
                                  name=f"qT{cb}") for cb in range(2)]
            kT_sb = [singles.tile([128, NT, 128], FP32R, tag=f"kT{cb}",
                                  name=f"kT{cb}") for cb in range(2)]
            aoT_sb = [singles.tile([128, NQ], FP32R, tag=f"aoT{cb}",
                                   name=f"aoT{cb}") for cb in range(2)]
            # stats layout per tile: [ksum, ksumsq, qsum, qsumsq]
            stats_sb = singles.tile([128, NT, 4], FP32)
            statsr_sb = singles.tile([128, NT, 4], FP32)
            mean_k = singles.tile([128, NT], FP32)
            rstd_k = singles.tile([128, NT], FP32)
            mean_q = singles.tile([128, NT], FP32)
            rstd_q = singles.tile([128, NT], FP32)

            def _allreduce_half(i):
                sl = slice(8 * i, 8 * (i + 1))
                nc.scalar.dma_start(
                    out=stats_dram[i][:, :],
                    in_=stats_sb[:, sl, :].rearrange("p t s -> p (t s)"))
                if MOCK_COLL:
                    nc.scalar.dma_start(out=statsr_dram[i][:, :],
                                        in_=stats_dram[i][:, :])
                else:
                    nc.gpsimd.collective_compute(
                        "AllReduce", mybir.AluOpType.add,
                        replica_groups=GROUPS,
                        ins=[stats_dram[i].ap().opt()],
                        outs=[statsr_dram[i].ap().opt()])
                nc.scalar.dma_start(
                    out=statsr_sb[:, sl, :].rearrange("p t s -> p (t s)"),
                    in_=statsr_dram[i][:, :])
                # mean/rstd for k and q over this half
                for (mean_t, rstd_t, c0) in ((mean_k, rstd_k, 0),
                                             (mean_q, rstd_q, 2)):
                    nc.vector.tensor_scalar_mul(
                        mean_t[:, sl], in0=statsr_sb[:, sl, c0], scalar1=1.0 / D)
                    var_t = work.tile([128, 8], FP32, tag="var", bufs=2)
                    nc.vector.tensor_scalar_mul(
                        var_t, in0=statsr_sb[:, sl, c0 + 1], scalar1=1.0 / D)
                    m2 = work.tile([128, 8], FP32, tag="m2", bufs=2)
                    nc.vector.tensor_mul(m2, mean_t[:, sl], mean_t[:, sl])
                    nc.vector.tensor_sub(var_t, var_t, m2)
                    nc.scalar.activation(var_t, var_t,
                                         mybir.ActivationFunctionType.Sqrt,
                                         bias=eps_sb)
                    nc.vector.reciprocal(rstd_t[:, sl], var_t)

            # --- stage A: kv-proj, k/q partial stats, v pack ---
            for t in range(NT):
                ctx_sb = ld.tile([128, KC, 128], FP32R, tag="ctx")
                nc.sync.dma_start(out=ctx_sb,
                                  in_=ctxT_r[:, :, 128 * t:128 * (t + 1)]
                                  .bitcast(FP32R))
                kv_ps = psmm.tile([128, 2 * DSL], FP32, tag="mm")
                for kk in range(KC):
                    nc.tensor.matmul(kv_ps, lhsT=ctx_sb[:, kk, :],
                                     rhs=wkv_sb[:, kk, :],
                                     start=(kk == 0), stop=(kk == KC - 1))
                nc.vector.tensor_copy(k_nat[:, t, :], kv_ps[:, 0:DSL])
                for h in range(HPC):
                    nc.scalar.copy(vh_sb[h][:, t, 0:DH],
                                   kv_ps[:, DSL + DH * h:DSL + DH * (h + 1)])
                nc.gpsimd.dma_start(out=x_nat[:, t, :],
                                    in_=x_s[128 * t:128 * (t + 1), :])
                # partial stats: sums on DVE, sumsq on Act (Square+accum)
                nc.vector.reduce_sum(out=stats_sb[:, t, 0:1],
                                     in_=k_nat[:, t, :],
                                     axis=mybir.AxisListType.X)
                scr = work.tile([128, DSL], FP32, tag="scr", bufs=2)
                nc.scalar.activation(scr, k_nat[:, t, :],
                                     mybir.ActivationFunctionType.Square,
                                     accum_out=stats_sb[:, t, 1:2])
                nc.vector.reduce_sum(out=stats_sb[:, t, 2:3],
                                     in_=x_nat[:, t, :],
                                     axis=mybir.AxisListType.X)
                scr2 = work.tile([128, DSL], FP32, tag="scr", bufs=2)
                nc.scalar.activation(scr2, x_nat[:, t, :],
                                     mybir.ActivationFunctionType.Square,
                                     accum_out=stats_sb[:, t, 3:4])
                if t == 7:
                    _allreduce_half(0)
            _allreduce_half(1)
            nc.gpsimd.dma_start(out=wout_sb, in_=woutT_r.bitcast(FP32R))

            def _prep_q_tile(tq):
                q_nat = work.tile([128, DSL], FP32, tag="qn")
                nc.vector.tensor_scalar(out=q_nat, in0=x_nat[:, tq, :],
                                        scalar1=mean_q[:, tq:tq + 1],
                                        scalar2=rstd_q[:, tq:tq + 1],
                                        op0=mybir.AluOpType.subtract,
                                        op1=mybir.AluOpType.mult)
                for cb in range(2):
                    tp = psmm.tile([128, 2 * DSL], FP32, tag="mm")
                    nc.tensor.transpose(tp[:, 0:128],
                                        q_nat[:, 128 * cb:128 * (cb + 1)],
                                        ident)
                    nc.vector.tensor_scalar(out=qT_sb[cb][:, tq, :],
                                            in0=tp[:, 0:128],
                                            scalar1=gqT[cb], scalar2=bqT[cb],
                                            op0=mybir.AluOpType.mult,
                                            op1=mybir.AluOpType.add)

            def _prep_k_tile(j):
                nc.vector.tensor_scalar(out=k_nat[:, j, :], in0=k_nat[:, j, :],
                                        scalar1=mean_k[:, j:j + 1],
                                        scalar2=rstd_k[:, j:j + 1],
                                        op0=mybir.AluOpType.subtract,
                                        op1=mybir.AluOpType.mult)
                for cb in range(2):
                    tp = psmm.tile([128, 2 * DSL], FP32, tag="mm")
                    nc.tensor.transpose(tp[:, 0:128],
                                        k_nat[:, j, 128 * cb:128 * (cb + 1)],
                                        ident)
                    nc.vector.tensor_scalar(out=kT_sb[cb][:, j, :],
                                            in0=tp[:, 0:128],
                                            scalar1=gkT[cb], scalar2=bkT[cb],
                                            op0=mybir.AluOpType.mult,
                                            op1=mybir.AluOpType.add)

            # --- attention + partial out-proj + per-block ReduceScatter ---
            for iblk in range(NBLK):
                for tq in range(TPB * iblk, TPB * (iblk + 1)):
                    _prep_q_tile(tq)
                for h in range(HPC):
                    cb, hh = h // 2, h % 2
                    qhT = qT_sb[cb][64 * hh:64 * (hh + 1),
                                    TPB * iblk:TPB * (iblk + 1), :]
                    oT_ps = psout.tile([DH + 2, 512], FP32, tag="oT")
                    for p in range(NT // 2):
                        j0, j1 = 2 * p, 2 * p + 1
                        if iblk == 0 and h == 0:
                            _prep_k_tile(j0)
                            _prep_k_tile(j1)
                        s_ps = pssim.tile([128, 1024], FP32, tag="sim")
                        nc.tensor.matmul(
                            s_ps[:, 0:512],
                            lhsT=kT_sb[cb][64 * hh:64 * (hh + 1), j0, :],
                            rhs=qhT, start=True, stop=True)
                        nc.tensor.matmul(
                            s_ps[:, 512:1024],
                            lhsT=kT_sb[cb][64 * hh:64 * (hh + 1), j1, :],
                            rhs=qhT, start=True, stop=True)
                        e_sb = work.tile([128, 1024], FP32R, tag="exp", bufs=4)
                        nc.scalar.activation(e_sb, s_ps,
                                             mybir.ActivationFunctionType.Exp,
                                             scale=SCALE)
                        nc.tensor.matmul(oT_ps, lhsT=vh_sb[h][:, j0, :],
                                         rhs=e_sb[:, 0:512],
                                         start=(p == 0), stop=False)
                        nc.tensor.matmul(oT_ps, lhsT=vh_sb[h][:, j1, :],
                                         rhs=e_sb[:, 512:1024],
                                         start=False, stop=(p == NT // 2 - 1))
                    # normalize: row DH holds the softmax denominators
                    csr1 = work.tile([1, 512], FP32, tag="csr1", bufs=2)
                    nc.vector.reciprocal(csr1, oT_ps[DH:DH + 1, :])
                    csr = work.tile([64, 512], FP32, tag="csr", bufs=2)
                    nc.gpsimd.partition_broadcast(csr, csr1)
                    nc.vector.tensor_mul(
                        aoT_sb[cb][64 * hh:64 * (hh + 1),
                                   512 * iblk:512 * (iblk + 1)],
                        oT_ps[0:DH, :], csr)
                # partial out-proj for this block: contract the core's 256
                # channels; team ReduceScatter sums partials
                for sub in range(4):
                    c0 = 512 * iblk + 128 * sub
                    y_sb = work.tile([128, D], FP32, tag="y", bufs=2)
                    for eb in range(2):
                        y_ps = psmm.tile([128, 2 * DSL], FP32, tag="mm")
                        for kk in range(KCO):
                            nc.tensor.matmul(
                                y_ps,
                                lhsT=aoT_sb[kk][:, c0:c0 + 128],
                                rhs=wout_sb[:, kk, 512 * eb:512 * (eb + 1)],
                                start=(kk == 0), stop=(kk == KCO - 1))
                        nc.vector.tensor_add(y_sb[:, 512 * eb:512 * (eb + 1)],
                                             y_ps,
                                             bout_b[:, 512 * eb:512 * (eb + 1)])
                    nc.sync.dma_start(
                        out=ypart_blk[iblk][128 * sub:128 * (sub + 1), :],
                        in_=y_sb)
                if MOCK_COLL:
                    nc.sync.dma_start(out=yrs_blk[iblk][:, :],
                                      in_=ypart_blk[iblk][0:128, :])
                else:
                    nc.gpsimd.collective_compute(
                        "ReduceScatter", mybir.AluOpType.add,
                        replica_groups=GROUPS,
                        ins=[ypart_blk[iblk].ap().opt()],
                        outs=[yrs_blk[iblk].ap().opt()])
                nc.gpsimd.dma_start(out=y_out[iblk, :, :],
                                    in_=yrs_blk[iblk][:, :])

    nc.finalize()
    return nc


def kernel(x, context, gq, bq, gk, bk, W_kv, W_out, b_out):
    x = np.asarray(x, dtype=np.float32)
    context = np.asarray(context, dtype=np.float32)
    gq = np.asarray(gq, dtype=np.float32)
    bq = np.asarray(bq, dtype=np.float32)
    gk = np.asarray(gk, dtype=np.float32)
    bk = np.asarray(bk, dtype=np.float32)
    W_kv = np.asarray(W_kv, dtype=np.float32)
    W_out = np.asarray(W_out, dtype=np.float32)
    b_out = np.asarray(b_out, dtype=np.float32)

    if "nc" not in _CACHE:
        _CACHE["nc"] = _build()
    nc = _CACHE["nc"]

    Wk, Wv = W_kv[:D], W_kv[D:]
    zeros_d = np.zeros((D,), np.float32)
    in_maps = []
    for c in range(NCORES):
        b, r = c // TEAM, c % TEAM
        sl = slice(DSL * r, DSL * (r + 1))
        wkvT_c = np.ascontiguousarray(
            np.concatenate([Wk[sl], Wv[sl]], axis=0).T)
        in_maps.append({
            "x_s": np.ascontiguousarray(x[b][:, sl]),
            "ctxT": np.ascontiguousarray(context[b].T),
            "wkvT": wkvT_c,
            "woutT": np.ascontiguousarray(W_out.T[sl, :]),
            "bout": b_out if r == 0 else zeros_d,
            "gq_s": np.ascontiguousarray(gq[sl]),
            "bq_s": np.ascontiguousarray(bq[sl]),
            "gk_s": np.ascontiguousarray(gk[sl]),
            "bk_s": np.ascontiguousarray(bk[sl]),
        })

    _CACHE["in_maps"] = in_maps
    try:
        res = run_bass_kernel_spmd(nc, in_maps, list(range(NCORES))).results
    except Exception:
        # transient runtime failures (device wedged from a prior run) --
        # one retry typically succeeds
        res = run_bass_kernel_spmd(nc, in_maps, list(range(NCORES))).results
    y = np.empty((B, NQ, D), dtype=np.float32)
    for c in range(NCORES):
        b, r = c // TEAM, c % TEAM
        for blk in range(NBLK):
            r0 = 512 * blk + 128 * r
            y[b, r0:r0 + 128, :] = res[c]["y_out"][blk]
    return y


# revision 6
# speedup vs baseline: 1.1787x; 1.1787x over previous
"""Cross-attention kernel for Trainium2, distributed over 8 NeuronCores.

Sharding: batch x head parallel. Cores 0-3 handle batch 0, cores 4-7 batch 1.
Within a team of 4, core r handles heads 4r..4r+3 (channel slice 256r..256r+256).

Per core:
  - KV projection for its 256 k-channels + 256 v-channels (tensor parallel,
    contraction over full D with host-pretransposed context/W_kv)
  - k and q LayerNorm stats: partial (sum, sumsq) per row over the core's
    256-channel slice; two 16KB AllReduces within the team (tiles 0-7 launch
    mid-projection so the latency hides under the rest of stage A)
  - k/q LN apply + transpose pipelined into the attention loop (per-tile,
    just before first use)
  - attention for its 4 heads, computed transposed (simT[j,i] = k.q) with
    softmax denominators from an appended ones-column in v (no max
    subtraction: |sim*scale| <= ~6 for this problem, exp stays in fp32 range)
  - Exp runs 1024 wide (two k-tiles per activation instruction); the AV
    matmuls are deferred by one iteration (software pipeline) so the PE
    queue always has independent sim work ahead of the exp-gated AV pair
    and the Activation engine never starves
  - output projection: partial y over the core's 256 channels for all 2048
    queries (W_out row-slice), per-512-row-block ReduceScatter within the
    team sums the partials; each core keeps a disjoint 128-row quarter of
    each block. Host assembles; bias is folded in by passing b_out to team
    rank 0 and zeros to ranks 1-3. Out-proj/q-prep/collectives are spread
    across the next block's head slots so block boundaries stay busy.
"""

import numpy as np

import concourse.bass as bass
import concourse.mybir as mybir
import concourse.tile as tile
from concourse import bacc
from concourse.bass_utils import run_bass_kernel_spmd
from concourse.masks import make_identity

B, NQ, NK, D, H, DH = 2, 2048, 2048, 1024, 16, 64
NCORES = 8
TEAM = 4
HPC = 4            # heads per core
DSL = HPC * DH     # 256: per-core channel slice
EPS = 1e-6
SCALE = DH ** -0.5
GROUPS = [[0, 1, 2, 3], [4, 5, 6, 7]]
FP32 = mybir.dt.float32
FP32R = mybir.dt.float32r
NT = NQ // 128     # 16 row tiles
KC = D // 128      # 8 contraction chunks (kv proj)
KCO = DSL // 128   # 2 contraction chunks (out proj)
NBLK = 4           # 512-query blocks
TPB = NT // NBLK   # 4 row tiles per block
NP = NT // 2       # 8 j-tile pairs per head

_CACHE: dict = {}
MOCK_COLL = False  # replace collectives with local DMA (for TimelineSim)


def _bcast_ap(t, parts):
    ap = t.ap() if hasattr(t, "ap") and not isinstance(t, bass.AP) else t
    return bass.AP(tensor=ap.tensor, offset=ap.offset,
                   ap=[[0, parts]] + list(ap.ap))


def _build():
    nc = bacc.Bacc("TRN2", target_bir_lowering=False, debug=False,
                   num_devices=NCORES)
    x_s = nc.declare_dram_parameter("x_s", [NQ, DSL], FP32, isOutput=False)
    ctxT = nc.declare_dram_parameter("ctxT", [D, NK], FP32, isOutput=False)
    wkvT = nc.declare_dram_parameter("wkvT", [D, 2 * DSL], FP32, isOutput=False)
    woutT = nc.declare_dram_parameter("woutT", [DSL, D], FP32, isOutput=False)
    bout = nc.declare_dram_parameter("bout", [D], FP32, isOutput=False)
    # packed LN params, columns: gq0 gq1 bq0 bq1 gk0 gk1 bk0 bk1
    pblob = nc.declare_dram_parameter("pblob", [128, 8], FP32, isOutput=False)
    y_out = nc.declare_dram_parameter("y_out", [NBLK, 128, D], FP32,
                                      isOutput=True)

    stats_dram = [nc.dram_tensor(f"stats_dram{i}", [128, 32], FP32)
                  for i in range(2)]
    statsr_dram = [nc.dram_tensor(f"statsr_dram{i}", [128, 32], FP32)
                   for i in range(2)]
    ypart_blk = [nc.dram_tensor(f"ypart_blk{i}", [512, D], FP32)
                 for i in range(NBLK)]
    yrs_blk = [nc.dram_tensor(f"yrs_blk{i}", [128, D], FP32)
               for i in range(NBLK)]

    ctxT_r = ctxT.ap().rearrange("(k p) m -> p k m", p=128)    # [128, 8, NK]
    wkvT_r = wkvT.ap().rearrange("(k p) n -> p k n", p=128)    # [128, 8, 512]
    woutT_r = woutT.ap().rearrange("(k p) n -> p k n", p=128)  # [128, 2, D]

    with tile.TileContext(nc) as tc:
        with (
            tc.tile_pool(name="singles", bufs=1) as singles,
            tc.tile_pool(name="ld", bufs=3) as ld,
            tc.tile_pool(name="work", bufs=3) as work,
            tc.tile_pool(name="psmm", bufs=2, space="PSUM") as psmm,
            tc.tile_pool(name="pssim", bufs=2, space="PSUM") as pssim,
            tc.tile_pool(name="psout", bufs=2, space="PSUM") as psout,
        ):
            # --- persistent sbuf ---
            pb_sb = singles.tile([128, 8], FP32)
            nc.sync.dma_start(out=pb_sb, in_=pblob.ap())
            wkv_sb = singles.tile([128, KC, 2 * DSL], FP32R)
            nc.scalar.dma_start(out=wkv_sb, in_=wkvT_r.bitcast(FP32R))
            ident = singles.tile([128, 128], FP32)
            make_identity(nc, ident)
            eps_sb = singles.tile([128, 1], FP32)
            nc.vector.memset(eps_sb, EPS)

            def _pcol(c):
                return pb_sb[:, c:c + 1]
            gqT = [_pcol(0), _pcol(1)]
            bqT = [_pcol(2), _pcol(3)]
            gkT = [_pcol(4), _pcol(5)]
            bkT = [_pcol(6), _pcol(7)]

            bout_b = singles.tile([128, D], FP32)
            wout_sb = singles.tile([128, KCO, D], FP32R)

            x_nat = singles.tile([128, NT, DSL], FP32)
            k_nat = singles.tile([128, NT, DSL], FP32)
            vh_sb = [singles.tile([128, NT, DH + 2], FP32R, tag=f"vh{h}",
                                  name=f"vh{h}") for h in range(HPC)]
            for h in range(HPC):
                nc.vector.memset(vh_sb[h][:, :, DH:DH + 1].bitcast(FP32), 1.0)
                nc.vector.memset(vh_sb[h][:, :, DH + 1:DH + 2].bitcast(FP32), 0.0)
            qT_sb = [singles.tile([128, NT, 128], FP32R, tag=f"qT{cb}",
                                  name=f"qT{cb}") for cb in range(2)]
            kT_sb = [singles.tile([128, NT, 128], FP32R, tag=f"kT{cb}",
                                  name=f"kT{cb}") for cb in range(2)]
            aoT_sb = [singles.tile([128, NQ], FP32R, tag=f"aoT{cb}",
                                   name=f"aoT{cb}") for cb in range(2)]
            # stats layout per tile: [ksum, ksumsq, qsum, qsumsq]
            stats_sb = singles.tile([128, NT, 4], FP32)
            statsr_sb = singles.tile([128, NT, 4], FP32)
            mean_k = singles.tile([128, NT], FP32)
            rstd_k = singles.tile([128, NT], FP32)
            mean_q = singles.tile([128, NT], FP32)
            rstd_q = singles.tile([128, NT], FP32)

            def _allreduce_half(i):
                sl = slice(8 * i, 8 * (i + 1))
                nc.scalar.dma_start(
                    out=stats_dram[i][:, :],
                    in_=stats_sb[:, sl, :].rearrange("p t s -> p (t s)"))
                if MOCK_COLL:
                    nc.scalar.dma_start(out=statsr_dram[i][:, :],
                                        in_=stats_dram[i][:, :])
                else:
                    nc.gpsimd.collective_compute(
                        "AllReduce", mybir.AluOpType.add,
                        replica_groups=GROUPS,
                        ins=[stats_dram[i].ap().opt()],
                        outs=[statsr_dram[i].ap().opt()])
                nc.scalar.dma_start(
                    out=statsr_sb[:, sl, :].rearrange("p t s -> p (t s)"),
                    in_=statsr_dram[i][:, :])
                for (mean_t, rstd_t, c0) in ((mean_k, rstd_k, 0),
                                             (mean_q, rstd_q, 2)):
                    nc.vector.tensor_scalar_mul(
                        mean_t[:, sl], in0=statsr_sb[:, sl, c0], scalar1=1.0 / D)
                    var_t = work.tile([128, 8], FP32, tag="var", bufs=2)
                    nc.vector.tensor_scalar_mul(
                        var_t, in0=statsr_sb[:, sl, c0 + 1], scalar1=1.0 / D)
                    m2 = work.tile([128, 8], FP32, tag="m2", bufs=2)
                    nc.vector.tensor_mul(m2, mean_t[:, sl], mean_t[:, sl])
                    nc.vector.tensor_sub(var_t, var_t, m2)
                    nc.scalar.activation(var_t, var_t,
                                         mybir.ActivationFunctionType.Sqrt,
                                         bias=eps_sb)
                    nc.vector.reciprocal(rstd_t[:, sl], var_t)

            # --- stage A: kv-proj, k/q partial stats, v pack ---
            for t in range(NT):
                ctx_sb = ld.tile([128, KC, 128], FP32R, tag="ctx")
                nc.sync.dma_start(out=ctx_sb,
                                  in_=ctxT_r[:, :, 128 * t:128 * (t + 1)]
                                  .bitcast(FP32R))
                kv_ps = psmm.tile([128, 2 * DSL], FP32, tag="mm")
                for kk in range(KC):
                    nc.tensor.matmul(kv_ps, lhsT=ctx_sb[:, kk, :],
                                     rhs=wkv_sb[:, kk, :],
                                     start=(kk == 0), stop=(kk == KC - 1))
                nc.vector.tensor_copy(k_nat[:, t, :], kv_ps[:, 0:DSL])
                for h in range(HPC):
                    nc.scalar.copy(vh_sb[h][:, t, 0:DH],
                                   kv_ps[:, DSL + DH * h:DSL + DH * (h + 1)])
                nc.gpsimd.dma_start(out=x_nat[:, t, :],
                                    in_=x_s[128 * t:128 * (t + 1), :])
                # partial stats: sums on DVE, sumsq on Act (Square+accum)
                nc.vector.reduce_sum(out=stats_sb[:, t, 0:1],
                                     in_=k_nat[:, t, :],
                                     axis=mybir.AxisListType.X)
                scr = work.tile([128, DSL], FP32, tag="scr", bufs=2)
                nc.scalar.activation(scr, k_nat[:, t, :],
                                     mybir.ActivationFunctionType.Square,
                                     accum_out=stats_sb[:, t, 1:2])
                nc.vector.reduce_sum(out=stats_sb[:, t, 2:3],
                                     in_=x_nat[:, t, :],
                                     axis=mybir.AxisListType.X)
                scr2 = work.tile([128, DSL], FP32, tag="scr", bufs=2)
                nc.scalar.activation(scr2, x_nat[:, t, :],
                                     mybir.ActivationFunctionType.Square,
                                     accum_out=stats_sb[:, t, 3:4])
                if t == 7:
                    _allreduce_half(0)
            _allreduce_half(1)
            # weights/bias needed from the first out-proj (end of block 0)
            nc.gpsimd.dma_start(out=wout_sb, in_=woutT_r.bitcast(FP32R))
            nc.gpsimd.dma_start(out=bout_b, in_=_bcast_ap(bout, 128))

            def _prep_q_tile(tq):
                q_nat = work.tile([128, DSL], FP32, tag="qn")
                nc.vector.tensor_scalar(out=q_nat, in0=x_nat[:, tq, :],
                                        scalar1=mean_q[:, tq:tq + 1],
                                        scalar2=rstd_q[:, tq:tq + 1],
                                        op0=mybir.AluOpType.subtract,
                                        op1=mybir.AluOpType.mult)
                for cb in range(2):
                    tp = psmm.tile([128, 2 * DSL], FP32, tag="mm")
                    nc.tensor.transpose(tp[:, 0:128],
                                        q_nat[:, 128 * cb:128 * (cb + 1)],
                                        ident)
                    nc.vector.tensor_scalar(out=qT_sb[cb][:, tq, :],
                                            in0=tp[:, 0:128],
                                            scalar1=gqT[cb], scalar2=bqT[cb],
                                            op0=mybir.AluOpType.mult,
                                            op1=mybir.AluOpType.add)

            def _prep_k_tile(j):
                nc.vector.tensor_scalar(out=k_nat[:, j, :], in0=k_nat[:, j, :],
                                        scalar1=mean_k[:, j:j + 1],
                                        scalar2=rstd_k[:, j:j + 1],
                                        op0=mybir.AluOpType.subtract,
                                        op1=mybir.AluOpType.mult)
                for cb in range(2):
                    tp = psmm.tile([128, 2 * DSL], FP32, tag="mm")
                    nc.tensor.transpose(tp[:, 0:128],
                                        k_nat[:, j, 128 * cb:128 * (cb + 1)],
                                        ident)
                    nc.vector.tensor_scalar(out=kT_sb[cb][:, j, :],
                                            in0=tp[:, 0:128],
                                            scalar1=gkT[cb], scalar2=bkT[cb],
                                            op0=mybir.AluOpType.mult,
                                            op1=mybir.AluOpType.add)

            def _party_sub(i, sub):
                c0 = 512 * i + 128 * sub
                y_sb = work.tile([128, D], FP32, tag="y", bufs=2)
                for eb in range(2):
                    y_ps = psmm.tile([128, 2 * DSL], FP32, tag="mm")
                    for kk in range(KCO):
                        nc.tensor.matmul(
                            y_ps[:, 0:512],
                            lhsT=aoT_sb[kk][:, c0:c0 + 128],
                            rhs=wout_sb[:, kk, 512 * eb:512 * (eb + 1)],
                            start=(kk == 0), stop=(kk == KCO - 1))
                    nc.vector.tensor_add(y_sb[:, 512 * eb:512 * (eb + 1)],
                                         y_ps[:, 0:512],
                                         bout_b[:, 512 * eb:512 * (eb + 1)])
                nc.sync.dma_start(
                    out=ypart_blk[i][128 * sub:128 * (sub + 1), :],
                    in_=y_sb)

            def _rs_block(i):
                if MOCK_COLL:
                    nc.sync.dma_start(out=yrs_blk[i][:, :],
                                      in_=ypart_blk[i][0:128, :])
                else:
                    nc.gpsimd.collective_compute(
                        "ReduceScatter", mybir.AluOpType.add,
                        replica_groups=GROUPS,
                        ins=[ypart_blk[i].ap().opt()],
                        outs=[yrs_blk[i].ap().opt()])
                nc.gpsimd.dma_start(out=y_out[i, :, :], in_=yrs_blk[i][:, :])

            def _normalize(oT_ps, i, h):
                cb, hh = h // 2, h % 2
                csr1 = work.tile([1, 512], FP32, tag="csr1", bufs=2)
                nc.vector.reciprocal(csr1, oT_ps[DH:DH + 1, :])
                csr = work.tile([64, 512], FP32, tag="csr", bufs=2)
                nc.gpsimd.partition_broadcast(csr, csr1)
                nc.vector.tensor_mul(
                    aoT_sb[cb][64 * hh:64 * (hh + 1),
                               512 * i:512 * (i + 1)],
                    oT_ps[0:DH, :], csr)

            # --- attention: software-pipelined over (block, head, j-pair) ---
            for tq in range(TPB):
                _prep_q_tile(tq)

            state = {"pend": None, "prev": None}

            def _emit_pend():
                if state["pend"] is None:
                    return
                oT_ps, h, p, e_sb, j0, j1 = state["pend"]
                nc.tensor.matmul(oT_ps, lhsT=vh_sb[h][:, j0, :],
                                 rhs=e_sb[:, 0:512],
                                 start=(p == 0), stop=False)
                nc.tensor.matmul(oT_ps, lhsT=vh_sb[h][:, j1, :],
                                 rhs=e_sb[:, 512:1024],
                                 start=False, stop=(p == NP - 1))
                state["pend"] = None

            for i in range(NBLK):
                for h in range(HPC):
                    cb, hh = h // 2, h % 2
                    qhT = qT_sb[cb][64 * hh:64 * (hh + 1),
                                    TPB * i:TPB * (i + 1), :]
                    oT_ps = psout.tile([DH + 2, 512], FP32, tag="oT")
                    for p in range(NP):
                        j0, j1 = 2 * p, 2 * p + 1
                        if i == 0 and h == 0:
                            _prep_k_tile(j0)
                            _prep_k_tile(j1)
                        s_ps = pssim.tile([128, 1024], FP32, tag="sim")
                        nc.tensor.matmul(
                            s_ps[:, 0:512],
                            lhsT=kT_sb[cb][64 * hh:64 * (hh + 1), j0, :],
                            rhs=qhT, start=True, stop=True)
                        nc.tensor.matmul(
                            s_ps[:, 512:1024],
                            lhsT=kT_sb[cb][64 * hh:64 * (hh + 1), j1, :],
                            rhs=qhT, start=True, stop=True)
                        _emit_pend()
                        e_sb = work.tile([128, 1024], FP32R, tag="exp", bufs=4)
                        nc.scalar.activation(e_sb, s_ps,
                                             mybir.ActivationFunctionType.Exp,
                                             scale=SCALE)
                        state["pend"] = (oT_ps, h, p, e_sb, j0, j1)
                    # post-p-loop slot work (previous head / previous block)
                    if state["prev"] is not None:
                        _normalize(*state["prev"])
                    state["prev"] = (oT_ps, i, h)
                    if h == 0 and i > 0:
                        _party_sub(i - 1, 0)
                        _party_sub(i - 1, 1)
                    elif h == 1 and i > 0:
                        _party_sub(i - 1, 2)
                        _party_sub(i - 1, 3)
                        _rs_block(i - 1)
                    elif h == 2 and i < NBLK - 1:
                        _prep_q_tile(TPB * (i + 1))
                        _prep_q_tile(TPB * (i + 1) + 1)
                    elif h == 3 and i < NBLK - 1:
                        _prep_q_tile(TPB * (i + 1) + 2)
                        _prep_q_tile(TPB * (i + 1) + 3)
            # drain the pipeline
            _emit_pend()
            _normalize(*state["prev"])
            for sub in range(4):
                _party_sub(NBLK - 1, sub)
            _rs_block(NBLK - 1)

    nc.finalize()
    return nc


def kernel(x, context, gq, bq, gk, bk, W_kv, W_out, b_out):
    x = np.asarray(x, dtype=np.float32)
    context = np.asarray(context, dtype=np.float32)
    gq = np.asarray(gq, dtype=np.float32)
    bq = np.asarray(bq, dtype=np.float32)
    gk = np.asarray(gk, dtype=np.float32)
    bk = np.asarray(bk, dtype=np.float32)
    W_kv = np.asarray(W_kv, dtype=np.float32)
    W_out = np.asarray(W_out, dtype=np.float32)
    b_out = np.asarray(b_out, dtype=np.float32)

    if "nc" not in _CACHE:
        _CACHE["nc"] = _build()
    nc = _CACHE["nc"]

    Wk, Wv = W_kv[:D], W_kv[D:]
    zeros_d = np.zeros((D,), np.float32)
    in_maps = []
    for c in range(NCORES):
        b, r = c // TEAM, c % TEAM
        sl = slice(DSL * r, DSL * (r + 1))
        wkvT_c = np.ascontiguousarray(
            np.concatenate([Wk[sl], Wv[sl]], axis=0).T)
        gqs, bqs, gks, bks = gq[sl], bq[sl], gk[sl], bk[sl]
        pb = np.stack([gqs[0:128], gqs[128:256], bqs[0:128], bqs[128:256],
                       gks[0:128], gks[128:256], bks[0:128], bks[128:256]],
                      axis=1)
        in_maps.append({
            "x_s": np.ascontiguousarray(x[b][:, sl]),
            "ctxT": np.ascontiguousarray(context[b].T),
            "wkvT": wkvT_c,
            "woutT": np.ascontiguousarray(W_out.T[sl, :]),
            "bout": b_out if r == 0 else zeros_d,
            "pblob": np.ascontiguousarray(pb),
        })

    _CACHE["in_maps"] = in_maps
    try:
        res = run_bass_kernel_spmd(nc, in_maps, list(range(NCORES))).results
    except Exception:
        # transient runtime failures (device wedged from a prior run) --
        # one retry typically succeeds
        res = run_bass_kernel_spmd(nc, in_maps, list(range(NCORES))).results
    y = np.empty((B, NQ, D), dtype=np.float32)
    for c in range(NCORES):
        b, r = c // TEAM, c % TEAM
        for blk in range(NBLK):
            r0 = 512 * blk + 128 * r
            y[b, r0:r0 + 128, :] = res[c]["y_out"][blk]
    return y
